# revision 3
# baseline (speedup 1.0000x reference)
"""Two-layer LSTM (B=256, T=256, D=128, H=1024, O=128) on 8 trn2 NeuronCores.

Strategy: 8-way tensor-parallel over the 4H gate dimension (as baseline), with
the collectives restructured to get off the recurrence critical path:

- TWO AllGathers per superstep (h0 right after layer0's activation, h1 right
  after layer1's), instead of one combined gather at superstep end.  Each is
  64KB/rank; consumers wait only on the gather they need.
- Layer1's W_ih matmuls depend only on the h0 gather; its W_hh matmuls only on
  the h1 gather, so the post-gather tensor segment is just 32 matmuls.
- Gathered h kept as 8 separate [128, B] chunk tiles per state; the 8 post-
  collective DMAs are fully contiguous on both sides and run on HWDGE queues
  (nc.sync, ~0.6us first-byte) rather than SWDGE (~2us).
- Gate biases folded into the scalar-engine activation (bias AP), removing the
  ones-matmul per gate (8 matmuls/superstep).
- PSUM packed 2 gates per bank ([128, 512] = i|f and g|o), double-buffered
  across supersteps: 8 banks exactly, so next-step x-part matmuls can start
  while the previous step's activations drain.
- Output bias applied on host; no ones tensor on device.
- v5: fp16 activation intermediates (less ACT->DVE traffic), x loads batched
  4 timesteps per DMA, deeper DRAM/act pools for scheduling slack.
"""

import numpy as np

import concourse.bass as bass
from concourse import bacc
import concourse.mybir as mybir
import concourse.tile as tile
from concourse.bass_utils import run_bass_kernel_spmd

B, T, D, H, O = 256, 256, 128, 1024, 128
NC = 8
HC = H // NC          # 128 h rows per core
GC = 4 * HC           # 512 gate rows per core
KH = H // 128         # 8 k-chunks over H
FP = mybir.dt.float16
F32 = mybir.dt.float32
AFT = mybir.ActivationFunctionType
RG = [list(range(NC))]


def _gate_dst(GA, GB, m):
    """psum target slice for gate m (0=i,1=f,2=g,3=o)."""
    t = GA if m < 2 else GB
    c = (m % 2) * B
    return t[:, c:c + B]


def _act_block(nc, apool, GA, GB, bT, c_sb, tag):
    """LSTM gate activations + cell update for one 128-slice.

    GA: psum [128, 2B] holding i|f pre-activations (no bias).
    GB: psum [128, 2B] holding g|o.  bT: bias tile [128, 4] f32.
    c_sb: fp32 cell tile [128, B], updated in place.
    Returns new h chunk [128, B] fp16."""
    ft = apool.tile([HC, B], FP, tag=tag + "f")
    it = apool.tile([HC, B], FP, tag=tag + "i")
    gt = apool.tile([HC, B], FP, tag=tag + "g")
    ot = apool.tile([HC, B], FP, tag=tag + "o")
    nc.scalar.activation(ft[:], GA[:, B:2 * B], AFT.Sigmoid, bias=bT[:, 1:2])
    nc.scalar.activation(it[:], GA[:, 0:B], AFT.Sigmoid, bias=bT[:, 0:1])
    nc.scalar.activation(gt[:], GB[:, 0:B], AFT.Tanh, bias=bT[:, 2:3])
    fc = apool.tile([HC, B], F32, tag=tag + "fc")
    ig = apool.tile([HC, B], F32, tag=tag + "ig")
    nc.vector.tensor_mul(fc[:], ft[:], c_sb[:])
    nc.vector.tensor_mul(ig[:], it[:], gt[:])
    nc.scalar.activation(ot[:], GB[:, B:2 * B], AFT.Sigmoid, bias=bT[:, 3:4])
    nc.vector.tensor_add(c_sb[:], fc[:], ig[:])
    tc_ = apool.tile([HC, B], FP, tag=tag + "tc")
    nc.scalar.activation(tc_[:], c_sb[:], AFT.Tanh)
    hnew = apool.tile([HC, B], FP, tag=tag + "h")
    nc.vector.tensor_mul(hnew[:], ot[:], tc_[:])
    return hnew


def build(t_steps):
    nc = bass.Bass(num_devices=NC)

    xT = nc.dram_tensor("xT", [t_steps, D, B], FP, kind="ExternalInput")
    w0i = nc.dram_tensor("w0i", [D, GC], FP, kind="ExternalInput")
    w0h = nc.dram_tensor("w0h", [KH, 128, GC], FP, kind="ExternalInput")
    w1i = nc.dram_tensor("w1i", [KH, 128, GC], FP, kind="ExternalInput")
    w1h = nc.dram_tensor("w1h", [KH, 128, GC], FP, kind="ExternalInput")
    wo = nc.dram_tensor("wo", [KH, 128, O], FP, kind="ExternalInput")
    b0T = nc.dram_tensor("b0T", [HC, 4], F32, kind="ExternalInput")
    b1T = nc.dram_tensor("b1T", [HC, 4], F32, kind="ExternalInput")
    h0T = nc.dram_tensor("h0T", [KH, 128, B], FP, kind="ExternalInput")
    c0T = nc.dram_tensor("c0T", [HC, B], F32, kind="ExternalInput")
    outT = nc.dram_tensor("outT", [O, B], F32, kind="ExternalOutput")

    with tile.TileContext(nc) as tc:
        with (
            tc.tile_pool(name="wpool", bufs=1) as wpool,
            tc.tile_pool(name="spool", bufs=1) as spool,
            tc.tile_pool(name="xpool", bufs=3) as xpool,
            tc.tile_pool(name="apool", bufs=3) as apool,
            tc.tile_pool(name="ppool", bufs=1, space="PSUM") as ppool,
            tc.tile_pool(name="dpool", bufs=3, space="DRAM") as dpool,
        ):
            w0i_sb = wpool.tile([D, GC], FP, tag="w0i")
            w0h_sb = wpool.tile([128, KH * GC], FP, tag="w0h")
            w1i_sb = wpool.tile([128, KH * GC], FP, tag="w1i")
            w1h_sb = wpool.tile([128, KH * GC], FP, tag="w1h")
            wo_sb = wpool.tile([128, KH * O], FP, tag="wo")
            b0_sb = wpool.tile([HC, 4], F32, tag="b0T")
            b1_sb = wpool.tile([HC, 4], F32, tag="b1T")
            nc.sync.dma_start(w0i_sb[:], w0i[:])
            nc.sync.dma_start(w0h_sb[:].rearrange("p (k m) -> p k m", k=KH), w0h[:].rearrange("k p m -> p k m"))
            nc.sync.dma_start(w1i_sb[:].rearrange("p (k m) -> p k m", k=KH), w1i[:].rearrange("k p m -> p k m"))
            nc.sync.dma_start(w1h_sb[:].rearrange("p (k m) -> p k m", k=KH), w1h[:].rearrange("k p m -> p k m"))
            nc.sync.dma_start(wo_sb[:].rearrange("p (k m) -> p k m", k=KH), wo[:].rearrange("k p m -> p k m"))
            nc.sync.dma_start(b0_sb[:], b0T[:])
            nc.sync.dma_start(b1_sb[:], b1T[:])

            # gathered-state chunk tiles: [parity][k] -> [128, B] fp16
            h0s = [[spool.tile([128, B], FP, tag=f"h0_{p}_{k}", name=f"h0_{p}_{k}")
                    for k in range(KH)] for p in (0, 1)]
            h1s = [[spool.tile([128, B], FP, tag=f"h1_{p}_{k}", name=f"h1_{p}_{k}")
                    for k in range(KH)] for p in (0, 1)]
            hini = [spool.tile([128, B], FP, tag=f"hi_{k}", name=f"hi_{k}") for k in range(KH)]
            for k in range(KH):
                nc.sync.dma_start(hini[k][:], h0T[k])
            c0_sb = spool.tile([HC, B], F32, tag="c0")
            c1_sb = spool.tile([HC, B], F32, tag="c1")
            nc.sync.dma_start(c0_sb[:], c0T[:])
            nc.sync.dma_start(c1_sb[:], c0T[:])

            # psum: 2 banks per layer per parity = 8 banks total
            GA0 = [ppool.tile([128, 2 * B], F32, tag=f"ga0_{p}", name=f"ga0_{p}") for p in (0, 1)]
            GB0 = [ppool.tile([128, 2 * B], F32, tag=f"gb0_{p}", name=f"gb0_{p}") for p in (0, 1)]
            GA1 = [ppool.tile([128, 2 * B], F32, tag=f"ga1_{p}", name=f"ga1_{p}") for p in (0, 1)]
            GB1 = [ppool.tile([128, 2 * B], F32, tag=f"gb1_{p}", name=f"gb1_{p}") for p in (0, 1)]

            for s in range(t_steps + 1):
                p = s % 2
                q = 1 - p
                if s < t_steps:
                    # ---- layer0, step s ----
                    if s % 4 == 0:
                        xt4 = xpool.tile([D, 4 * B], FP, tag="xt")
                        nb = min(4, t_steps - s)
                        nc.sync.dma_start(
                            xt4[:, :nb * B].rearrange("d (t b) -> d t b", t=nb),
                            xT[s:s + nb].rearrange("t d b -> d t b"))
                    xt = xt4[:, (s % 4) * B:(s % 4 + 1) * B]
                    ga, gb = GA0[p], GB0[p]
                    for m in range(4):
                        # one start per PSUM bank: m=0 clears GA, m=2 clears GB
                        nc.tensor.matmul(
                            _gate_dst(ga, gb, m),
                            w0i_sb[:, m * 128:(m + 1) * 128], xt,
                            start=(m % 2 == 0), stop=False, skip_group_check=True)
                    h0cur = h0s[p] if s >= 1 else hini
                    for pair in (0, 1):  # (i,f) bank first, then (g,o)
                        for k in range(KH):
                            for m in (2 * pair, 2 * pair + 1):
                                nc.tensor.matmul(
                                    _gate_dst(ga, gb, m),
                                    w0h_sb[:, k * GC + m * 128: k * GC + (m + 1) * 128],
                                    h0cur[k][:],
                                    start=False,
                                    stop=(k == KH - 1 and m % 2 == 1),
                                    skip_group_check=True)
                    h0new = _act_block(nc, apool, ga, gb, b0_sb, c0_sb, "l0")
                    # all-gather h0new
                    cc0i = dpool.tile([HC, B], FP, tag="cc0i")
                    cc0o = dpool.tile([NC, HC, B], FP, tag="cc0o")
                    nc.sync.dma_start(cc0i[:], h0new[:])
                    nc.gpsimd.collective_compute(
                        "AllGather", mybir.AluOpType.bypass, replica_groups=RG,
                        ins=[cc0i.opt()], outs=[cc0o.opt()])
                    for k in range(KH):
                        nc.sync.dma_start(h0s[q][k][:], cc0o[k])
                if s >= 1:
                    # ---- layer1, step s-1 ----
                    ga, gb = GA1[p], GB1[p]
                    h0cur = h0s[p]
                    h1cur = h1s[p] if s >= 2 else hini
                    for pair in (0, 1):
                        for k in range(KH):
                            for m in (2 * pair, 2 * pair + 1):
                                nc.tensor.matmul(
                                    _gate_dst(ga, gb, m),
                                    w1i_sb[:, k * GC + m * 128: k * GC + (m + 1) * 128],
                                    h0cur[k][:],
                                    start=(k == 0 and m % 2 == 0), stop=False,
                                    skip_group_check=True)
                    for pair in (0, 1):
                        for k in range(KH):
                            for m in (2 * pair, 2 * pair + 1):
                                nc.tensor.matmul(
                                    _gate_dst(ga, gb, m),
                                    w1h_sb[:, k * GC + m * 128: k * GC + (m + 1) * 128],
                                    h1cur[k][:],
                                    start=False,
                                    stop=(k == KH - 1 and m % 2 == 1),
                                    skip_group_check=True)
                    h1new = _act_block(nc, apool, ga, gb, b1_sb, c1_sb, "l1")
                    cc1i = dpool.tile([HC, B], FP, tag="cc1i")
                    cc1o = dpool.tile([NC, HC, B], FP, tag="cc1o")
                    nc.sync.dma_start(cc1i[:], h1new[:])
                    nc.gpsimd.collective_compute(
                        "AllGather", mybir.AluOpType.bypass, replica_groups=RG,
                        ins=[cc1i.opt()], outs=[cc1o.opt()])
                    for k in range(KH):
                        nc.sync.dma_start(h1s[q][k][:], cc1o[k])

            # output projection: out^T[O, B] = W_out @ h1_T^T (bias on host)
            pfin = (t_steps + 1) % 2
            po = ppool.tile([O, B], F32, tag="ga0_0")
            for k in range(KH):
                nc.tensor.matmul(
                    po[:], wo_sb[:, k * O:(k + 1) * O],
                    h1s[pfin][k][:],
                    start=(k == 0), stop=(k == KH - 1))
            out_sb = apool.tile([O, B], F32, tag="out")
            nc.scalar.copy(out_sb[:], po[:])
            nc.sync.dma_start(outT[:], out_sb[:])

    _split_excess_waits(nc)
    return nc


def _split_excess_waits(nc):
    """This walrus build embeds at most ONE sync wait per instruction (any
    type).  Move excess waits onto same-engine drains inserted immediately
    before the instruction, one wait per drain — engine queues execute in
    order, so semantics are unchanged."""
    for bb in nc.main_func.blocks:
        insts = list(bb.instructions)
        inserts = {}
        extras = []
        for pos, ins in enumerate(insts):
            si = ins.sync_info
            if si is None or not si.on_wait or len(si.on_wait) <= 1:
                continue
            waits = list(si.on_wait)
            keep, excess = waits[-1:], waits[:-1]
            carriers = []
            for w in excess:
                d = nc.engines[ins.engine].drain(fusable=False).ins
                d.sync_info = mybir.SyncInfo(on_wait=[w], on_update=[])
                carriers.append(d)
                extras.append(d)
            inserts[pos] = carriers
            si.on_wait = keep
            ins.sync_info = si
        if not inserts:
            continue
        extra_set = set(id(e) for e in extras)
        for blk in nc.main_func.blocks:
            blk.instructions = [i for i in blk.instructions
                                if id(i) not in extra_set]
        out = []
        for pos, ins in enumerate(insts):
            out.extend(inserts.get(pos, ()))
            out.append(ins)
        bb.instructions = out


def make_in_maps(x, h0, c0, W_ih0, W_hh0, b_ih0, b_hh0,
                 W_ih1, W_hh1, b_ih1, b_hh1, W_out, b_out, t_steps):
    xT = np.ascontiguousarray(
        np.transpose(x[:, :t_steps, :], (1, 2, 0))).astype(np.float16)
    h0T_full = np.ascontiguousarray(h0.T).astype(np.float16).reshape(KH, 128, B)
    c0T_full = np.ascontiguousarray(c0.T).astype(np.float32)
    wo_host = np.ascontiguousarray(W_out.T).astype(np.float16).reshape(KH, 128, O)
    in_maps = []
    for j in range(NC):
        idx = np.concatenate(
            [np.arange(g * H + j * HC, g * H + (j + 1) * HC) for g in range(4)])
        w0i_j = np.ascontiguousarray(W_ih0[idx].T).astype(np.float16)
        w0h_j = np.ascontiguousarray(W_hh0[idx].T).astype(np.float16).reshape(KH, 128, GC)
        w1i_j = np.ascontiguousarray(W_ih1[idx].T).astype(np.float16).reshape(KH, 128, GC)
        w1h_j = np.ascontiguousarray(W_hh1[idx].T).astype(np.float16).reshape(KH, 128, GC)
        b0_j = np.ascontiguousarray(
            (b_ih0 + b_hh0)[idx].reshape(4, HC).T).astype(np.float32)
        b1_j = np.ascontiguousarray(
            (b_ih1 + b_hh1)[idx].reshape(4, HC).T).astype(np.float32)
        in_maps.append({
            "xT": xT, "w0i": w0i_j, "w0h": w0h_j, "w1i": w1i_j, "w1h": w1h_j,
            "wo": wo_host, "b0T": b0_j, "b1T": b1_j,
            "h0T": h0T_full, "c0T": c0T_full[j * HC:(j + 1) * HC],
        })
    return in_maps


def run(t_steps, in_maps, trace=False):
    nc = build(t_steps)
    res = run_bass_kernel_spmd(nc, in_maps, list(range(NC)), trace=trace)
    return res


def kernel(**inputs):
    args = {k: np.asarray(v) for k, v in inputs.items()}
    in_maps = make_in_maps(
        args["x"], args["h0"], args["c0"],
        args["W_ih0"], args["W_hh0"], args["b_ih0"], args["b_hh0"],
        args["W_ih1"], args["W_hh1"], args["b_ih1"], args["b_hh1"],
        args["W_out"], args["b_out"], T)
    res = run(T, in_maps)
    outT = res.results[0]["outT"] + args["b_out"].astype(np.float32)[:, None]
    return np.ascontiguousarray(outT.T).astype(np.float32)
